# revision 12
# baseline (speedup 1.0000x reference)
"""Trainium2 Bass kernel for EnhancedPathAwareECA.

Data-parallel over batch: 16 examples split as 2 per NeuronCore across 8 cores
(no collectives — per-example stats are local). The op is memory-bound
elementwise scaling (out = x * per-(b,p,d) scale), so HBM traffic is the whole
game: x is cast to fp16 on the host and the output is written fp16 then upcast
on the host, halving both directions vs f32 (rel-err ~3e-4, far under the
2e-2 gate). Per core: 16 MiB in + 16 MiB out at ~358 GB/s => ~95 us floor.

Schedule (per core, 2 examples x 8 path-tiles of [128, 4096] fp16):
- All 16 x-tiles are SBUF-resident (16 MiB) — loads never wait on stores.
- Per-path sum over l is split across two engines per tile: a DVE fp16
  pair-add (2x mode, ~1.1 us) halves the columns into a scratch tile, then an
  ACT in-place Copy with fp32 accum_out (~1.9 us) finishes the sum. Any DVE
  accumulation runs at 1x (accumulator readback steals a read port), so
  tensor_reduce/tensor_scalar+accum on the full tile (4.3 us) is avoided.
- Stats chain (9-tap conv over d, sigmoid attn, LayerNorm over paths, erf-gelu
  gate MLP) stays fp32; weight folds (combined conv taps with 1/l, ln_g/D,
  b1/sqrt(2)) happen on the host. Sqrt forces 2 ACT table swaps/example;
  they mostly hide under concurrent DVE/PE chain ops.
- Drain of example e is interleaved pairwise with the sums of example e+1 on
  DVE ([mul_p, pair-add_p] pairs) so stores start right after stats while the
  next example's sums stay load-paced.
- Loads own the sync HWDGE ring; stores ride the scalar (ACT) ring (the only
  other HWDGE ring) so they never head-of-line-block loads. The last
  example's drain alternates both rings.
- Scale multiplies: in-place fp16 tensor_scalar_mul (4x mode, ~1.2 us) with
  the f32 per-path scalar from SBUF.
"""

import sys
from contextlib import ExitStack

import numpy as np

sys.path.insert(0, "/opt/trn_rl_repo")

N_CORES = 8
B, C, L = 16, 1024, 4096
P, D = 8, 128            # paths, dims per path (C = P*D)
BLOC = B // N_CORES      # examples per core
LN_EPS = 1e-5
H = L // 2

_cached = None


def _build():
    import concourse.tile as tile
    from concourse import bacc, masks, mybir

    f16 = mybir.dt.float16
    f32 = mybir.dt.float32
    OP = mybir.AluOpType
    AF = mybir.ActivationFunctionType

    nc = bacc.Bacc(
        "TRN2",
        target_bir_lowering=False,
        debug=False,
        num_devices=N_CORES,
    )

    x_in = nc.dram_tensor("x_local", [BLOC, C, L], f16, kind="ExternalInput")
    a9_d = nc.dram_tensor("a9", [P, 9], f32, kind="ExternalInput")
    cb_d = nc.dram_tensor("cb8", [P, 1], f32, kind="ExternalInput")
    lng_d = nc.dram_tensor("lng", [P, 1], f32, kind="ExternalInput")
    lnb_d = nc.dram_tensor("lnb", [P, 1], f32, kind="ExternalInput")
    w1_d = nc.dram_tensor("w1", [P, 2 * P], f32, kind="ExternalInput")
    b1_d = nc.dram_tensor("b1t", [2 * P, 1], f32, kind="ExternalInput")
    w2_d = nc.dram_tensor("w2", [2 * P, P], f32, kind="ExternalInput")
    b2_d = nc.dram_tensor("b2t", [P, 1], f32, kind="ExternalInput")
    b1e_d = nc.dram_tensor("b1e", [2 * P, 1], f32, kind="ExternalInput")
    y_out = nc.dram_tensor("y_local", [BLOC, C, L], f16, kind="ExternalOutput")

    x_ap = x_in.ap()
    y_ap = y_out.ap()

    with tile.TileContext(nc) as tc, ExitStack() as ctx:
        consts = ctx.enter_context(tc.tile_pool(name="consts", bufs=1))
        xp = ctx.enter_context(tc.tile_pool(name="xp", bufs=BLOC * P))
        sp = ctx.enter_context(tc.tile_pool(name="sp", bufs=4))
        sm = ctx.enter_context(tc.tile_pool(name="sm", bufs=2))
        pp = ctx.enter_context(tc.tile_pool(name="pp", bufs=1, space="PSUM"))

        def cload(dram, shape):
            # consts ride the ACT ring so x loads start immediately on sync
            t = consts.tile(shape, f32, name=dram.name + "_sb", tag=dram.name)
            nc.scalar.dma_start(out=t[:], in_=dram.ap()[:, :])
            return t

        ident = consts.tile([128, 128], f32)
        masks.make_identity(nc, ident[:])
        ones18 = consts.tile([1, P], f32)
        nc.vector.memset(ones18[:], 1.0)
        # warm the sigmoid/erf ACT table during the preamble so the first
        # stats chain doesn't pay the initial table load
        warm = consts.tile([1, 1], f32)
        nc.vector.memset(warm[:], 0.0)
        nc.scalar.activation(out=warm[:], in_=warm[:], func=AF.Sigmoid)

        xts = [[None] * P for _ in range(BLOC)]
        ysums = [None] * BLOC

        def load_tile(e, p):
            xt = xp.tile([128, L], f16, tag="x", name=f"x_{e}_{p}")
            csl = slice(p * 128, (p + 1) * 128)
            nc.sync.dma_start(out=xt[:], in_=x_ap[e, csl, 0:L])
            xts[e][p] = xt
            return xt

        def accum(scr, e, col, gate):
            # ACT finishes a sum with its fp32 accumulator; `gate` (exact
            # ones derived from the previous chain's output, via scale=) keeps
            # the scheduler from pulling these into the chain's ACT queue.
            if gate is None:
                nc.scalar.activation(
                    out=scr[:], in_=scr[:], func=AF.Copy,
                    accum_out=ysums[e][:, col:col + 1])
            else:
                nc.scalar.activation(
                    out=scr[:], in_=scr[:], func=AF.Copy, scale=gate[:],
                    accum_out=ysums[e][:, col:col + 1])

        def pair_add(xt_or_scr, lo, w, e, tag):
            # DVE fp16 pair-add at 2x mode: [*, lo:lo+w] -> [*, w/2]
            scr = sp.tile([128, w // 2], f16, tag="scr", name=tag)
            nc.vector.tensor_tensor(
                out=scr[:], in0=xt_or_scr[:, lo:lo + w // 2],
                in1=xt_or_scr[:, lo + w // 2:lo + w], op=OP.add)
            return scr

        def sum_tile(e, p, gate=None):
            # Early paths (p<4) reduce 3 levels on DVE (its idle window)
            # down to [128,512] so their ACT accum costs only ~0.8us; late
            # paths stop at one level (accum@2048); the last path splits into
            # two independent half-sums (extra ysum col, combined in
            # stats_chain) so the final accum lands ~1.4us after its load.
            xt = xts[e][p]
            if p == P - 1:
                for sl, col in ((slice(0, H), p), (slice(H, L), P)):
                    scr = pair_add(xt, sl.start, H, e, f"scrh_{e}_{col}")
                    accum(scr, e, col, gate)
                return
            scr = pair_add(xt, 0, L, e, f"scr_{e}_{p}")
            if p < 4:
                scr = pair_add(scr, 0, H, e, f"scr2_{e}_{p}")
                scr = pair_add(scr, 0, H // 2, e, f"scr3_{e}_{p}")
            accum(scr, e, p, gate)

        def stats_chain(e):
            """ysums[e] [d, p] f32 -> scaleT [d, p] f32 (attn * gate)."""
            ys = ysums[e]
            nc.vector.tensor_add(ys[:, P - 1:P], ys[:, P - 1:P], ys[:, P:P + 1])
            ysum_ps = pp.tile([P, D], f32, tag="ysum_ps", bufs=2)
            nc.tensor.transpose(ysum_ps[:], ys[:, 0:P], ident[:])

            # combined 9-tap grouped conv along d (zero-padded)
            ypad = sm.tile([P, D + 8], f32, tag="ypad")
            nc.vector.memset(ypad[:, 0:4], 0.0)
            nc.vector.memset(ypad[:, D + 4:D + 8], 0.0)
            nc.vector.tensor_copy(ypad[:, 4:D + 4], ysum_ps[:])
            acc = [sm.tile([P, D], f32, tag=f"acc{i}", name=f"acc{i}_{e}")
                   for i in range(2)]
            nc.vector.tensor_scalar_mul(acc[0][:], ypad[:, 0:D], a9[:, 0:1])
            cur = 0
            for k in range(1, 9):
                nxt = 1 - cur
                nc.vector.scalar_tensor_tensor(
                    out=acc[nxt][:], in0=ypad[:, k:k + D], scalar=a9[:, k:k + 1],
                    in1=acc[cur][:], op0=OP.mult, op1=OP.add)
                cur = nxt

            # attn = sigmoid(logit + combine_b); crosssum = sum_d attn
            attn = sm.tile([P, D], f32, tag="attn")
            rhs2 = sm.tile([P, 2], f32, tag="rhs2")  # [ones | crosssum]
            nc.vector.memset(rhs2[:, 0:1], 1.0)
            nc.scalar.activation(out=attn[:], in_=acc[cur][:], func=AF.Sigmoid,
                                 bias=cb8[:], accum_out=rhs2[:, 1:2])

            # LayerNorm over the 8 paths (crosssum units; 1/D folded)
            stats_ps = pp.tile([1, 2], f32, tag="stats")  # [sum, sumsq]
            nc.tensor.matmul(stats_ps[:], rhs2[:, 1:2], rhs2[:], start=True,
                             stop=True)
            musig = sm.tile([1, 2], f32, tag="musig")     # [mu_s, rstd]
            nc.vector.tensor_scalar_mul(musig[:, 0:1], stats_ps[:, 0:1], 1.0 / P)
            musq = sm.tile([1, 1], f32, tag="musq")
            nc.vector.tensor_mul(musq[:], musig[:, 0:1], musig[:, 0:1])
            var_s = sm.tile([1, 1], f32, tag="var_s")
            nc.vector.scalar_tensor_tensor(
                out=var_s[:], in0=stats_ps[:, 1:2], scalar=1.0 / P, in1=musq[:],
                op0=OP.mult, op1=OP.subtract)
            den2 = sm.tile([1, 1], f32, tag="den2")
            nc.vector.tensor_scalar(
                out=den2[:], in0=var_s[:], scalar1=1.0 / (D * D), scalar2=LN_EPS,
                op0=OP.mult, op1=OP.add)
            denom = sm.tile([1, 1], f32, tag="denom")
            nc.scalar.sqrt(denom[:], den2[:])
            nc.vector.reciprocal(musig[:, 1:2], denom[:])
            bc_ps = pp.tile([P, 2], f32, tag="bc")        # broadcast mu/rstd
            nc.tensor.matmul(bc_ps[:], ones18[:], musig[:], start=True, stop=True)
            t8 = sm.tile([P, 1], f32, tag="t8")
            nc.vector.scalar_tensor_tensor(
                out=t8[:], in0=rhs2[:, 1:2], scalar=bc_ps[:, 0:1],
                in1=bc_ps[:, 1:2], op0=OP.subtract, op1=OP.mult)
            h8 = sm.tile([P, 1], f32, tag="h8")
            nc.vector.scalar_tensor_tensor(
                out=h8[:], in0=t8[:], scalar=lng[:], in1=lnb[:],
                op0=OP.mult, op1=OP.add)

            # gate MLP: sigmoid(W2.T gelu(W1.T h + b1) + b2); exact erf-gelu
            # (Erf shares the Sigmoid ACT table — no extra swap)
            z1_ps = pp.tile([2 * P, 1], f32, tag="z1")
            nc.tensor.matmul(z1_ps[:], w1[:], h8[:], start=True, stop=True)
            e16 = sm.tile([2 * P, 1], f32, tag="e16")
            nc.scalar.activation(out=e16[:], in_=z1_ps[:], func=AF.Erf,
                                 scale=0.7071067811865476, bias=b1e[:])
            z1b = sm.tile([2 * P, 1], f32, tag="z1b")
            nc.vector.tensor_scalar_add(z1b[:], z1_ps[:], b1t[:])
            e1p = sm.tile([2 * P, 1], f32, tag="e1p")
            nc.vector.tensor_scalar_add(e1p[:], e16[:], 1.0)
            h1t = sm.tile([2 * P, 1], f32, tag="h1t")
            nc.vector.scalar_tensor_tensor(
                out=h1t[:], in0=z1b[:], scalar=0.5, in1=e1p[:],
                op0=OP.mult, op1=OP.mult)
            z2_ps = pp.tile([P, 1], f32, tag="z2")
            nc.tensor.matmul(z2_ps[:], w2[:], h1t[:], start=True, stop=True)
            gatet = sm.tile([P, 1], f32, tag="gatet")
            nc.scalar.activation(out=gatet[:], in_=z2_ps[:], func=AF.Sigmoid,
                                 bias=b2t[:])

            # scale = attn * gate, transposed to [d, p]
            scale8 = sm.tile([P, D], f32, tag="scale8")
            nc.vector.tensor_scalar_mul(scale8[:], attn[:], gatet[:])
            scaleT_ps = pp.tile([128, P], f32, tag="scaleT", bufs=2)
            nc.tensor.transpose(scaleT_ps[:], scale8[:], ident[0:P, 0:P])
            scaleT = sm.tile([128, P], f32, tag="scaleT_sb",
                             name=f"scaleT_{e}")
            nc.vector.tensor_copy(scaleT[:], scaleT_ps[:])
            return scaleT

        def mul_store(e, p, scaleT, ring, chunks=1):
            # chunks>1 fine-grains the first tile after a stats seam so store
            # bytes start flowing ~2us sooner
            xt = xts[e][p]
            csl = slice(p * 128, (p + 1) * 128)
            cw = L // chunks
            for c in range(chunks):
                sl = slice(c * cw, (c + 1) * cw)
                nc.vector.tensor_scalar_mul(
                    xt[:, sl], xt[:, sl], scaleT[:, p:p + 1])
                ring.dma_start(out=y_ap[e, csl, sl], in_=xt[:, sl])

        # ---- phase A: example-0 load + sums ----
        ysums[0] = sm.tile([128, P + 1], f32, tag="ysumT", name="ysum_0")
        for p in range(P):
            load_tile(0, p)
            if p == 0:
                # consts dispatch behind the first x load; they ride the
                # scalar ring and land long before the stats chain needs them
                a9 = cload(a9_d, [P, 9])
                cb8 = cload(cb_d, [P, 1])
                lng = cload(lng_d, [P, 1])
                lnb = cload(lnb_d, [P, 1])
                w1 = cload(w1_d, [P, 2 * P])
                b1t = cload(b1_d, [2 * P, 1])
                w2 = cload(w2_d, [2 * P, P])
                b2t = cload(b2_d, [P, 1])
                b1e = cload(b1e_d, [2 * P, 1])
            sum_tile(0, p)
        # ---- phase B: example-1 loads enqueue on the sync ring now ----
        ysums[1] = sm.tile([128, P + 1], f32, tag="ysumT", name="ysum_1")
        for p in range(P):
            load_tile(1, p)
        # ---- phase C: example-0 stats ----
        scaleT0 = stats_chain(0)
        # Accum gate: e1's ACT accums only become ready once the e0 chain is
        # done, so they cannot jam its ACT queue (the chain gates the gscale
        # release below).
        zb1 = sm.tile([128, 1], f32, tag="zb", name="zb1")
        nc.vector.tensor_scalar(out=zb1[:], in0=scaleT0[:, 0:1], scalar1=0.0,
                                scalar2=1.0, op0=OP.mult, op1=OP.add)
        # Drain gate: hold e0's muls (and so its stores) until e1's 7th tile
        # has loaded. Loads then run at full read bandwidth with no store
        # competition, e1's sums/chain start as early as possible, and e0's
        # stores become a dense block that bridges e1's stats seam. The gate
        # is a ones vector data-dependent on both scaleT0 and the e1 p6 load,
        # folded into the scale operand (exact multiply by 1.0).
        zg = sm.tile([128, 1], f32, tag="zg")
        nc.vector.tensor_scalar(out=zg[:], in0=scaleT0[:, 0:1], scalar1=0.0,
                                scalar2=1.0, op0=OP.mult, op1=OP.add)
        zg2 = sm.tile([128, 1], f32, tag="zg2")
        nc.vector.scalar_tensor_tensor(
            out=zg2[:], in0=xts[1][6][:, 0:1], scalar=0.0, in1=zg[:],
            op0=OP.mult, op1=OP.add)
        gscaleT0 = sm.tile([128, P], f32, tag="gscaleT0")
        nc.vector.tensor_scalar_mul(gscaleT0[:], scaleT0[:], zg2[:])
        # ---- phase D: e1 sums (tile-paced accums post-chain) + gated e0
        # drain ----
        # muls lead their paired sum: early sums pre-hoist into DVE's idle
        # window, so the gated muls flow back-to-back at gate release and
        # store bytes hit the pipe the moment loads finish
        for p in range(P):
            mul_store(0, p, gscaleT0, nc.scalar, chunks=4 if p == 0 else 1)
            sum_tile(1, p, gate=zb1)
        # ---- phase E: example-1 stats ----
        scaleT1 = stats_chain(1)
        # ---- phase F: e1 drain on both rings (no loads left on sync) ----
        for p in range(P - 1):
            mul_store(1, p, scaleT1, nc.sync if p % 2 == 0 else nc.scalar,
                      chunks=4 if p == 0 else 1)
        # last tile: halves on both rings so the final transfer tail halves
        xt = xts[1][P - 1]
        csl = slice((P - 1) * 128, P * 128)
        nc.vector.tensor_scalar_mul(xt[:, 0:H], xt[:, 0:H],
                                    scaleT1[:, P - 1:P])
        nc.scalar.dma_start(out=y_ap[1, csl, 0:H], in_=xt[:, 0:H])
        nc.vector.tensor_scalar_mul(xt[:, H:L], xt[:, H:L],
                                    scaleT1[:, P - 1:P])
        nc.sync.dma_start(out=y_ap[1, csl, H:L], in_=xt[:, H:L])

    nc.compile()
    return nc


def _get_nc():
    global _cached
    if _cached is None:
        _cached = _build()
    return _cached


def _fold_weights(conv1_w, conv2_w, combine_w, combine_b, ln_g, ln_b, W1, b1, W2, b2):
    a9 = np.zeros((P, 9), np.float32)
    a9[:, 2:7] += combine_w[0] * conv1_w
    a9[:, :] += combine_w[1] * conv2_w
    a9 /= L  # fold mean over l into the conv taps
    return {
        "a9": np.ascontiguousarray(a9),
        "cb8": np.full((P, 1), float(combine_b), np.float32),
        "lng": np.ascontiguousarray((ln_g / D).reshape(P, 1).astype(np.float32)),
        "lnb": np.ascontiguousarray(ln_b.reshape(P, 1).astype(np.float32)),
        "w1": np.ascontiguousarray(W1.astype(np.float32)),
        "b1t": np.ascontiguousarray(b1.reshape(2 * P, 1).astype(np.float32)),
        "w2": np.ascontiguousarray(W2.astype(np.float32)),
        "b2t": np.ascontiguousarray(b2.reshape(P, 1).astype(np.float32)),
        "b1e": np.ascontiguousarray(
            (b1 / np.sqrt(2.0)).reshape(2 * P, 1).astype(np.float32)),
    }


def run(x, consts, trace=False, **trace_kwargs):
    from concourse.bass_utils import run_bass_kernel_spmd

    nc = _get_nc()
    x16 = np.ascontiguousarray(x.astype(np.float16))
    core_ids = list(range(N_CORES))
    in_maps = []
    for i in core_ids:
        m = {"x_local": np.ascontiguousarray(x16[i * BLOC:(i + 1) * BLOC])}
        m.update(consts)
        in_maps.append(m)
    try:
        res = run_bass_kernel_spmd(nc, in_maps, core_ids, trace=trace,
                                   **trace_kwargs)
    except Exception:
        # transient NRT_EXEC_UNIT_UNRECOVERABLE after recompiles — one retry
        res = run_bass_kernel_spmd(nc, in_maps, core_ids, trace=trace,
                                   **trace_kwargs)
    out = np.concatenate(
        [res.results[i]["y_local"] for i in core_ids], axis=0
    ).astype(np.float32)
    return out, res


def kernel(x, conv1_w, conv2_w, combine_w, combine_b, ln_g, ln_b, W1, b1, W2, b2):
    x = np.asarray(x, np.float32)
    assert x.shape == (B, C, L), x.shape
    consts = _fold_weights(
        np.asarray(conv1_w, np.float32), np.asarray(conv2_w, np.float32),
        np.asarray(combine_w, np.float32), np.asarray(combine_b, np.float32),
        np.asarray(ln_g, np.float32), np.asarray(ln_b, np.float32),
        np.asarray(W1, np.float32), np.asarray(b1, np.float32),
        np.asarray(W2, np.float32), np.asarray(b2, np.float32))
    out, _ = run(x, consts)
    return out


# revision 13
# speedup vs baseline: 1.0062x; 1.0062x over previous
"""Trainium2 Bass kernel for EnhancedPathAwareECA.

Data-parallel over batch: 16 examples split as 2 per NeuronCore across 8 cores
(no collectives — per-example stats are local). The op is memory-bound
elementwise scaling (out = x * per-(b,p,d) scale), so HBM traffic is the whole
game: x is cast to fp16 on the host and the output is written fp16 then upcast
on the host, halving both directions vs f32 (rel-err ~3e-4, far under the
2e-2 gate). Per core: 16 MiB in + 16 MiB out at ~358 GB/s => ~95 us floor.

Schedule (per core, 2 examples x 8 path-tiles of [128, 4096] fp16):
- All 16 x-tiles are SBUF-resident (16 MiB) — loads never wait on stores.
- Per-path sum over l is split across two engines per tile: a DVE fp16
  pair-add (2x mode, ~1.1 us) halves the columns into a scratch tile, then an
  ACT in-place Copy with fp32 accum_out (~1.9 us) finishes the sum. Any DVE
  accumulation runs at 1x (accumulator readback steals a read port), so
  tensor_reduce/tensor_scalar+accum on the full tile (4.3 us) is avoided.
- Stats chain (9-tap conv over d, sigmoid attn, LayerNorm over paths, erf-gelu
  gate MLP) stays fp32; weight folds (combined conv taps with 1/l, ln_g/D,
  b1/sqrt(2)) happen on the host. Sqrt forces 2 ACT table swaps/example;
  they mostly hide under concurrent DVE/PE chain ops.
- Drain of example e is interleaved pairwise with the sums of example e+1 on
  DVE ([mul_p, pair-add_p] pairs) so stores start right after stats while the
  next example's sums stay load-paced.
- Loads own the sync HWDGE ring; stores ride the scalar (ACT) ring (the only
  other HWDGE ring) so they never head-of-line-block loads. The last
  example's drain alternates both rings.
- Scale multiplies: in-place fp16 tensor_scalar_mul (4x mode, ~1.2 us) with
  the f32 per-path scalar from SBUF.
"""

import sys
from contextlib import ExitStack

import numpy as np

sys.path.insert(0, "/opt/trn_rl_repo")

N_CORES = 8
B, C, L = 16, 1024, 4096
P, D = 8, 128            # paths, dims per path (C = P*D)
BLOC = B // N_CORES      # examples per core
LN_EPS = 1e-5
H = L // 2

_cached = None


def _build():
    import concourse.tile as tile
    from concourse import bacc, masks, mybir

    f16 = mybir.dt.float16
    f32 = mybir.dt.float32
    OP = mybir.AluOpType
    AF = mybir.ActivationFunctionType

    nc = bacc.Bacc(
        "TRN2",
        target_bir_lowering=False,
        debug=False,
        num_devices=N_CORES,
    )

    x_in = nc.dram_tensor("x_local", [BLOC, C, L], f16, kind="ExternalInput")
    a9_d = nc.dram_tensor("a9", [P, 9], f32, kind="ExternalInput")
    cb_d = nc.dram_tensor("cb8", [P, 1], f32, kind="ExternalInput")
    lng_d = nc.dram_tensor("lng", [P, 1], f32, kind="ExternalInput")
    lnb_d = nc.dram_tensor("lnb", [P, 1], f32, kind="ExternalInput")
    w1_d = nc.dram_tensor("w1", [P, 2 * P], f32, kind="ExternalInput")
    b1_d = nc.dram_tensor("b1t", [2 * P, 1], f32, kind="ExternalInput")
    w2_d = nc.dram_tensor("w2", [2 * P, P], f32, kind="ExternalInput")
    b2_d = nc.dram_tensor("b2t", [P, 1], f32, kind="ExternalInput")
    b1e_d = nc.dram_tensor("b1e", [2 * P, 1], f32, kind="ExternalInput")
    y_out = nc.dram_tensor("y_local", [BLOC, C, L], f16, kind="ExternalOutput")

    x_ap = x_in.ap()
    y_ap = y_out.ap()

    with tile.TileContext(nc) as tc, ExitStack() as ctx:
        consts = ctx.enter_context(tc.tile_pool(name="consts", bufs=1))
        xp = ctx.enter_context(tc.tile_pool(name="xp", bufs=BLOC * P))
        sp = ctx.enter_context(tc.tile_pool(name="sp", bufs=6))
        sm = ctx.enter_context(tc.tile_pool(name="sm", bufs=2))
        pp = ctx.enter_context(tc.tile_pool(name="pp", bufs=1, space="PSUM"))

        def cload(dram, shape):
            # consts ride the ACT ring so x loads start immediately on sync
            t = consts.tile(shape, f32, name=dram.name + "_sb", tag=dram.name)
            nc.scalar.dma_start(out=t[:], in_=dram.ap()[:, :])
            return t

        ident = consts.tile([128, 128], f32)
        masks.make_identity(nc, ident[:])
        ones18 = consts.tile([1, P], f32)
        nc.vector.memset(ones18[:], 1.0)
        # warm the sigmoid/erf ACT table during the preamble so the first
        # stats chain doesn't pay the initial table load
        warm = consts.tile([1, 1], f32)
        nc.vector.memset(warm[:], 0.0)
        nc.scalar.activation(out=warm[:], in_=warm[:], func=AF.Sigmoid)

        xts = [[None] * P for _ in range(BLOC)]
        ysums = [None] * BLOC

        def load_tile(e, p):
            xt = xp.tile([128, L], f16, tag="x", name=f"x_{e}_{p}")
            csl = slice(p * 128, (p + 1) * 128)
            nc.sync.dma_start(out=xt[:], in_=x_ap[e, csl, 0:L])
            xts[e][p] = xt
            return xt

        def accum(scr, e, col, gate):
            # ACT finishes a sum with its fp32 accumulator; `gate` (exact
            # ones derived from the previous chain's output, via scale=) keeps
            # the scheduler from pulling these into the chain's ACT queue.
            if gate is None:
                nc.scalar.activation(
                    out=scr[:], in_=scr[:], func=AF.Copy,
                    accum_out=ysums[e][:, col:col + 1])
            else:
                nc.scalar.activation(
                    out=scr[:], in_=scr[:], func=AF.Copy, scale=gate[:],
                    accum_out=ysums[e][:, col:col + 1])

        def pair_add(xt_or_scr, lo, w, e, tag):
            # DVE fp16 pair-add at 2x mode: [*, lo:lo+w] -> [*, w/2].
            # Tag per output width: each level gets its own pool-slot
            # rotation, so a pair-add never stalls waiting for a slot that
            # an unrelated accum hasn't released yet.
            scr = sp.tile([128, w // 2], f16, tag=f"scr{w // 2}", name=tag)
            nc.vector.tensor_tensor(
                out=scr[:], in0=xt_or_scr[:, lo:lo + w // 2],
                in1=xt_or_scr[:, lo + w // 2:lo + w], op=OP.add)
            return scr

        def sum_tile(e, p, gate=None):
            # Early paths (p<4) reduce 3 levels on DVE (its idle window)
            # down to [128,512] so their ACT accum costs only ~0.8us; late
            # paths stop at one level (accum@2048); the last path splits into
            # two independent half-sums (extra ysum col, combined in
            # stats_chain) so the final accum lands ~1.4us after its load.
            xt = xts[e][p]
            if p == P - 1:
                for sl, col in ((slice(0, H), p), (slice(H, L), P)):
                    scr = pair_add(xt, sl.start, H, e, f"scrh_{e}_{col}")
                    accum(scr, e, col, gate)
                return
            scr = pair_add(xt, 0, L, e, f"scr_{e}_{p}")
            if p < 4:
                scr = pair_add(scr, 0, H, e, f"scr2_{e}_{p}")
                scr = pair_add(scr, 0, H // 2, e, f"scr3_{e}_{p}")
            accum(scr, e, p, gate)

        def stats_chain(e):
            """ysums[e] [d, p] f32 -> scaleT [d, p] f32 (attn * gate)."""
            ys = ysums[e]
            nc.vector.tensor_add(ys[:, P - 1:P], ys[:, P - 1:P], ys[:, P:P + 1])
            ysum_ps = pp.tile([P, D], f32, tag="ysum_ps", bufs=2)
            nc.tensor.transpose(ysum_ps[:], ys[:, 0:P], ident[:])

            # combined 9-tap grouped conv along d (zero-padded)
            ypad = sm.tile([P, D + 8], f32, tag="ypad")
            nc.vector.memset(ypad[:, 0:4], 0.0)
            nc.vector.memset(ypad[:, D + 4:D + 8], 0.0)
            nc.vector.tensor_copy(ypad[:, 4:D + 4], ysum_ps[:])
            acc = [sm.tile([P, D], f32, tag=f"acc{i}", name=f"acc{i}_{e}")
                   for i in range(2)]
            nc.vector.tensor_scalar_mul(acc[0][:], ypad[:, 0:D], a9[:, 0:1])
            cur = 0
            for k in range(1, 9):
                nxt = 1 - cur
                nc.vector.scalar_tensor_tensor(
                    out=acc[nxt][:], in0=ypad[:, k:k + D], scalar=a9[:, k:k + 1],
                    in1=acc[cur][:], op0=OP.mult, op1=OP.add)
                cur = nxt

            # attn = sigmoid(logit + combine_b); crosssum = sum_d attn
            attn = sm.tile([P, D], f32, tag="attn")
            rhs2 = sm.tile([P, 2], f32, tag="rhs2")  # [ones | crosssum]
            nc.vector.memset(rhs2[:, 0:1], 1.0)
            nc.scalar.activation(out=attn[:], in_=acc[cur][:], func=AF.Sigmoid,
                                 bias=cb8[:], accum_out=rhs2[:, 1:2])

            # LayerNorm over the 8 paths (crosssum units; 1/D folded)
            stats_ps = pp.tile([1, 2], f32, tag="stats")  # [sum, sumsq]
            nc.tensor.matmul(stats_ps[:], rhs2[:, 1:2], rhs2[:], start=True,
                             stop=True)
            musig = sm.tile([1, 2], f32, tag="musig")     # [mu_s, rstd]
            nc.vector.tensor_scalar_mul(musig[:, 0:1], stats_ps[:, 0:1], 1.0 / P)
            musq = sm.tile([1, 1], f32, tag="musq")
            nc.vector.tensor_mul(musq[:], musig[:, 0:1], musig[:, 0:1])
            var_s = sm.tile([1, 1], f32, tag="var_s")
            nc.vector.scalar_tensor_tensor(
                out=var_s[:], in0=stats_ps[:, 1:2], scalar=1.0 / P, in1=musq[:],
                op0=OP.mult, op1=OP.subtract)
            den2 = sm.tile([1, 1], f32, tag="den2")
            nc.vector.tensor_scalar(
                out=den2[:], in0=var_s[:], scalar1=1.0 / (D * D), scalar2=LN_EPS,
                op0=OP.mult, op1=OP.add)
            denom = sm.tile([1, 1], f32, tag="denom")
            nc.scalar.sqrt(denom[:], den2[:])
            nc.vector.reciprocal(musig[:, 1:2], denom[:])
            bc_ps = pp.tile([P, 2], f32, tag="bc")        # broadcast mu/rstd
            nc.tensor.matmul(bc_ps[:], ones18[:], musig[:], start=True, stop=True)
            t8 = sm.tile([P, 1], f32, tag="t8")
            nc.vector.scalar_tensor_tensor(
                out=t8[:], in0=rhs2[:, 1:2], scalar=bc_ps[:, 0:1],
                in1=bc_ps[:, 1:2], op0=OP.subtract, op1=OP.mult)
            h8 = sm.tile([P, 1], f32, tag="h8")
            nc.vector.scalar_tensor_tensor(
                out=h8[:], in0=t8[:], scalar=lng[:], in1=lnb[:],
                op0=OP.mult, op1=OP.add)

            # gate MLP: sigmoid(W2.T gelu(W1.T h + b1) + b2); exact erf-gelu
            # (Erf shares the Sigmoid ACT table — no extra swap)
            z1_ps = pp.tile([2 * P, 1], f32, tag="z1")
            nc.tensor.matmul(z1_ps[:], w1[:], h8[:], start=True, stop=True)
            e16 = sm.tile([2 * P, 1], f32, tag="e16")
            nc.scalar.activation(out=e16[:], in_=z1_ps[:], func=AF.Erf,
                                 scale=0.7071067811865476, bias=b1e[:])
            z1b = sm.tile([2 * P, 1], f32, tag="z1b")
            nc.vector.tensor_scalar_add(z1b[:], z1_ps[:], b1t[:])
            e1p = sm.tile([2 * P, 1], f32, tag="e1p")
            nc.vector.tensor_scalar_add(e1p[:], e16[:], 1.0)
            h1t = sm.tile([2 * P, 1], f32, tag="h1t")
            nc.vector.scalar_tensor_tensor(
                out=h1t[:], in0=z1b[:], scalar=0.5, in1=e1p[:],
                op0=OP.mult, op1=OP.mult)
            z2_ps = pp.tile([P, 1], f32, tag="z2")
            nc.tensor.matmul(z2_ps[:], w2[:], h1t[:], start=True, stop=True)
            gatet = sm.tile([P, 1], f32, tag="gatet")
            nc.scalar.activation(out=gatet[:], in_=z2_ps[:], func=AF.Sigmoid,
                                 bias=b2t[:])

            # scale = attn * gate, transposed to [d, p]
            scale8 = sm.tile([P, D], f32, tag="scale8")
            nc.vector.tensor_scalar_mul(scale8[:], attn[:], gatet[:])
            scaleT_ps = pp.tile([128, P], f32, tag="scaleT", bufs=2)
            nc.tensor.transpose(scaleT_ps[:], scale8[:], ident[0:P, 0:P])
            scaleT = sm.tile([128, P], f32, tag="scaleT_sb",
                             name=f"scaleT_{e}")
            nc.vector.tensor_copy(scaleT[:], scaleT_ps[:])
            return scaleT

        def mul_store(e, p, scaleT, ring, chunks=1):
            # chunks>1 fine-grains the first tile after a stats seam so store
            # bytes start flowing ~2us sooner
            xt = xts[e][p]
            csl = slice(p * 128, (p + 1) * 128)
            cw = L // chunks
            for c in range(chunks):
                sl = slice(c * cw, (c + 1) * cw)
                nc.vector.tensor_scalar_mul(
                    xt[:, sl], xt[:, sl], scaleT[:, p:p + 1])
                ring.dma_start(out=y_ap[e, csl, sl], in_=xt[:, sl])

        # ---- phase A: example-0 load + sums ----
        ysums[0] = sm.tile([128, P + 1], f32, tag="ysumT", name="ysum_0")
        for p in range(P):
            load_tile(0, p)
            if p == 0:
                # consts dispatch behind the first x load; they ride the
                # scalar ring and land long before the stats chain needs them
                a9 = cload(a9_d, [P, 9])
                cb8 = cload(cb_d, [P, 1])
                lng = cload(lng_d, [P, 1])
                lnb = cload(lnb_d, [P, 1])
                w1 = cload(w1_d, [P, 2 * P])
                b1t = cload(b1_d, [2 * P, 1])
                w2 = cload(w2_d, [2 * P, P])
                b2t = cload(b2_d, [P, 1])
                b1e = cload(b1e_d, [2 * P, 1])
            sum_tile(0, p)
        # ---- phase B: example-1 loads enqueue on the sync ring now ----
        ysums[1] = sm.tile([128, P + 1], f32, tag="ysumT", name="ysum_1")
        for p in range(P):
            load_tile(1, p)
        # ---- phase C: example-0 stats ----
        scaleT0 = stats_chain(0)
        # Accum gate: e1's ACT accums only become ready once the e0 chain is
        # done, so they cannot jam its ACT queue (the chain gates the gscale
        # release below).
        zb1 = sm.tile([128, 1], f32, tag="zb", name="zb1")
        nc.vector.tensor_scalar(out=zb1[:], in0=scaleT0[:, 0:1], scalar1=0.0,
                                scalar2=1.0, op0=OP.mult, op1=OP.add)
        # Drain gate: hold e0's muls (and so its stores) until e1's 7th tile
        # has loaded. Loads then run at full read bandwidth with no store
        # competition, e1's sums/chain start as early as possible, and e0's
        # stores become a dense block that bridges e1's stats seam. The gate
        # is a ones vector data-dependent on both scaleT0 and the e1 p6 load,
        # folded into the scale operand (exact multiply by 1.0).
        zg = sm.tile([128, 1], f32, tag="zg")
        nc.vector.tensor_scalar(out=zg[:], in0=scaleT0[:, 0:1], scalar1=0.0,
                                scalar2=1.0, op0=OP.mult, op1=OP.add)
        zg2 = sm.tile([128, 1], f32, tag="zg2")
        nc.vector.scalar_tensor_tensor(
            out=zg2[:], in0=xts[1][6][:, 0:1], scalar=0.0, in1=zg[:],
            op0=OP.mult, op1=OP.add)
        gscaleT0 = sm.tile([128, P], f32, tag="gscaleT0")
        nc.vector.tensor_scalar_mul(gscaleT0[:], scaleT0[:], zg2[:])
        # ---- phase D: e1 sums (tile-paced accums post-chain) + gated e0
        # drain ----
        # muls lead their paired sum: early sums pre-hoist into DVE's idle
        # window, so the gated muls flow back-to-back at gate release and
        # store bytes hit the pipe the moment loads finish
        for p in range(P):
            mul_store(0, p, gscaleT0, nc.scalar, chunks=4 if p == 0 else 1)
            sum_tile(1, p, gate=zb1)
        # ---- phase E: example-1 stats ----
        scaleT1 = stats_chain(1)
        # ---- phase F: e1 drain on both rings (no loads left on sync) ----
        for p in range(P - 1):
            mul_store(1, p, scaleT1, nc.sync if p % 2 == 0 else nc.scalar,
                      chunks=4 if p == 0 else 1)
        # last tile: halves on both rings so the final transfer tail halves
        xt = xts[1][P - 1]
        csl = slice((P - 1) * 128, P * 128)
        nc.vector.tensor_scalar_mul(xt[:, 0:H], xt[:, 0:H],
                                    scaleT1[:, P - 1:P])
        nc.scalar.dma_start(out=y_ap[1, csl, 0:H], in_=xt[:, 0:H])
        nc.vector.tensor_scalar_mul(xt[:, H:L], xt[:, H:L],
                                    scaleT1[:, P - 1:P])
        nc.sync.dma_start(out=y_ap[1, csl, H:L], in_=xt[:, H:L])

    nc.compile()
    return nc


def _get_nc():
    global _cached
    if _cached is None:
        _cached = _build()
    return _cached


def _fold_weights(conv1_w, conv2_w, combine_w, combine_b, ln_g, ln_b, W1, b1, W2, b2):
    a9 = np.zeros((P, 9), np.float32)
    a9[:, 2:7] += combine_w[0] * conv1_w
    a9[:, :] += combine_w[1] * conv2_w
    a9 /= L  # fold mean over l into the conv taps
    return {
        "a9": np.ascontiguousarray(a9),
        "cb8": np.full((P, 1), float(combine_b), np.float32),
        "lng": np.ascontiguousarray((ln_g / D).reshape(P, 1).astype(np.float32)),
        "lnb": np.ascontiguousarray(ln_b.reshape(P, 1).astype(np.float32)),
        "w1": np.ascontiguousarray(W1.astype(np.float32)),
        "b1t": np.ascontiguousarray(b1.reshape(2 * P, 1).astype(np.float32)),
        "w2": np.ascontiguousarray(W2.astype(np.float32)),
        "b2t": np.ascontiguousarray(b2.reshape(P, 1).astype(np.float32)),
        "b1e": np.ascontiguousarray(
            (b1 / np.sqrt(2.0)).reshape(2 * P, 1).astype(np.float32)),
    }


def run(x, consts, trace=False, **trace_kwargs):
    from concourse.bass_utils import run_bass_kernel_spmd

    nc = _get_nc()
    x16 = np.ascontiguousarray(x.astype(np.float16))
    core_ids = list(range(N_CORES))
    in_maps = []
    for i in core_ids:
        m = {"x_local": np.ascontiguousarray(x16[i * BLOC:(i + 1) * BLOC])}
        m.update(consts)
        in_maps.append(m)
    try:
        res = run_bass_kernel_spmd(nc, in_maps, core_ids, trace=trace,
                                   **trace_kwargs)
    except Exception:
        # transient NRT_EXEC_UNIT_UNRECOVERABLE after recompiles — one retry
        res = run_bass_kernel_spmd(nc, in_maps, core_ids, trace=trace,
                                   **trace_kwargs)
    out = np.concatenate(
        [res.results[i]["y_local"] for i in core_ids], axis=0
    ).astype(np.float32)
    return out, res


def kernel(x, conv1_w, conv2_w, combine_w, combine_b, ln_g, ln_b, W1, b1, W2, b2):
    x = np.asarray(x, np.float32)
    assert x.shape == (B, C, L), x.shape
    consts = _fold_weights(
        np.asarray(conv1_w, np.float32), np.asarray(conv2_w, np.float32),
        np.asarray(combine_w, np.float32), np.asarray(combine_b, np.float32),
        np.asarray(ln_g, np.float32), np.asarray(ln_b, np.float32),
        np.asarray(W1, np.float32), np.asarray(b1, np.float32),
        np.asarray(W2, np.float32), np.asarray(b2, np.float32))
    out, _ = run(x, consts)
    return out
